# revision 10
# baseline (speedup 1.0000x reference)
"""AttentionPooling (segment softmax-weighted pooling) on 8 TRN2 NeuronCores.

Math (reference):
    h = relu(x @ W1 + b1)            # [N, 128]
    s = h @ W2 + b2                  # [N, 1]
    w = softmax(s, axis=0)           # over ALL nodes
    out[b] = sum_{i: batch[i]==b} w[i] * x[i]     # [512, 256]

Distribution: batch is sorted, so nodes are sharded at segment boundaries
(64 contiguous segments per core).  Each core computes exp(s_i) for its
nodes and a weighted segment-sum of x into its 64 segments plus
per-segment exp-sums (one matmul with a one-hot*exp stationary operand).
No on-device collective: every core returns its raw weighted sums and
exp-sums ([64, 257]); the host sums the 8 exp-sum columns into the global
softmax denominator and normalizes while unsharding.  Softmax is computed
unshifted (exp(s) without max subtraction): scores for this MLP are O(1),
far inside fp32 range.

Device layouts (all DMA streams partition-major so each partition reads
one long contiguous run — ~390 GB/s vs ~270 for 512B-chunk patterns):
    xnat_pm [p, t*257]   fp16  row t*128+p of [x | 1]  -> seg matmul moving
    xt_pm   [p, c, n]    fp16/fp8  x^T feature c*128+p -> L1 moving
    L1: h^T[h_p, node_f] = W1[d,h].T-stationary @ x_t          (PSUM f32)
    L2: s[node_p, 1]     = hr^T[h, node128]-stationary @ W2[h,1]
    seg: acc[seg_p, 257_f] += onehot_w[node, seg]-stationary @ [x_nat | 1]
         (col 256 accumulates per-segment exp-sums)
"""

import numpy as np

from concourse import bacc, mybir, tile
from concourse import bass_utils
from concourse.mybir import AluOpType, ActivationFunctionType as AFT

P = 128          # partitions / nodes per tile
D = 256          # feature dim
H = 128          # hidden dim
B = 512          # total segments
NCORES = 8
SEGS = B // NCORES   # segments per core
GROUP = 512      # nodes per L1 matmul group
SUPER = 4096     # nodes per DMA batch
STRIDE = D + 1   # x_nat SBUF row block: 256 features + ones column

F32 = mybir.dt.float32
F16 = mybir.dt.float16
F8 = mybir.dt.float8e4
I16 = mybir.dt.int16

NP_F8 = mybir.dt.np(F8)

# x_t chunk dtypes per precision mode (chunk0 = features 0:128, chunk1 =
# features 128:256).  fp8 halves that chunk's DMA bytes at ~1.1e-2 extra
# relative error on the softmax scores.
XT_MODES = {
    "f16": (F16, F16),
    "h128": (F16, F8),
    "f8": (F8, F8),
}

_cache: dict[tuple, object] = {}


def _build(nshard: int, loop: int = 1, mode: str = "f16",
           dma_only: bool = False, no_tail: bool = False):
    key = (nshard, loop, mode, dma_only, no_tail)
    if key in _cache:
        return _cache[key]
    assert nshard % GROUP == 0
    ntiles = nshard // P
    cdt = XT_MODES[mode]

    nc = bacc.Bacc("TRN2", target_bir_lowering=False, debug=False,
                   num_devices=NCORES)

    x_nat = nc.dram_tensor("x_nat", [P, ntiles * STRIDE], F16,
                           kind="ExternalInput")
    xt0 = nc.dram_tensor("xt0", [P, nshard], cdt[0], kind="ExternalInput")
    xt1 = nc.dram_tensor("xt1", [P, nshard], cdt[1], kind="ExternalInput")
    bloc = nc.dram_tensor("bloc", [P, ntiles], F32, kind="ExternalInput")
    w1 = nc.dram_tensor("w1", [D, H], F16, kind="ExternalInput")
    w2 = nc.dram_tensor("w2", [H, 1], F16, kind="ExternalInput")
    b1c = nc.dram_tensor("b1c", [H, 1], F32, kind="ExternalInput")
    b2c = nc.dram_tensor("b2c", [P, 1], F32, kind="ExternalInput")
    pooled = nc.dram_tensor("pooled", [SEGS, STRIDE], F32,
                            kind="ExternalOutput")

    with tile.TileContext(nc) as tc:
        with (
            tc.tile_pool(name="const", bufs=1) as const,
            tc.tile_pool(name="xn_pool", bufs=4) as xn_pool,
            tc.tile_pool(name="xt0_pool", bufs=4) as xt0_pool,
            tc.tile_pool(name="xt1_pool", bufs=4) as xt1_pool,
            tc.tile_pool(name="hr_pool", bufs=4) as hr_pool,
            tc.tile_pool(name="e_pool", bufs=3) as e_pool,
            tc.tile_pool(name="tail", bufs=1) as tail,
            tc.tile_pool(name="hp_pool", bufs=3, space="PSUM") as hp_pool,
            tc.tile_pool(name="sp_pool", bufs=3, space="PSUM") as sp_pool,
            tc.tile_pool(name="acc_pool", bufs=1, space="PSUM") as acc_pool,
        ):
            # ---- constants ----
            iota_i = const.tile([P, SEGS], I16)
            nc.gpsimd.iota(iota_i[:], pattern=[[1, SEGS]], base=0,
                           channel_multiplier=0)
            seg_iota = const.tile([P, SEGS], F16)
            nc.vector.tensor_copy(seg_iota[:], iota_i[:])

            # W1 [256,128] stored as [128, 2*H]: two K-chunks side by side
            w1t = const.tile([P, 2 * H], F16, name="w1t2")
            nc.scalar.dma_start(
                w1t[:].rearrange("p (c h) -> p c h", c=2),
                w1.ap().rearrange("(c p) h -> p c h", c=2))
            w2t = const.tile([H, 1], F16)
            nc.scalar.dma_start(w2t[:], w2.ap())
            b1t = const.tile([H, 1], F32)
            nc.scalar.dma_start(b1t[:], b1c.ap())
            b2t = const.tile([P, 1], F32)
            nc.scalar.dma_start(b2t[:], b2c.ap())
            bloc_t = const.tile([P, ntiles], F32)
            nc.scalar.dma_start(bloc_t[:], bloc.ap())

            # persistent accumulator: [seg, 256 features + expsum col].
            # 128 partitions (rows 64-127 accumulate zeros) so every matmul
            # in the kernel has identical M=128 PE geometry — a tile_size
            # switch (M=64 <-> M=128) between back-to-back matmuls costs
            # ~110ns/tile in array reconfiguration (HW-measured).
            acc = acc_pool.tile([P, STRIDE], F32)

            # one-hot ring: [128, 128] stationaries whose right 64 columns
            # stay zero (memset once); DVE only rewrites the left halves
            OH_SLOTS = 12
            oh_ring = const.tile([P, OH_SLOTS * 2 * SEGS], F16,
                                 name="oh_ring")
            nc.vector.memset(oh_ring[:], 0.0)
            oh_slot = [0]

            supers = []
            pos = 0
            taper_from = max(0, nshard - SUPER)
            while pos < nshard:
                limit = SUPER if pos < taper_from else GROUP
                sn = min(limit, nshard - pos)
                supers.append((pos, sn))
                pos += sn

            # Software-pipelined 3-stage emission so the PE instruction
            # stream never blocks on same-group ACT/DVE results (a >100ns
            # PE gap drops it from the 2.4 GHz p-state to 1.2 GHz):
            #   A(g):  L1 matmuls + relu            (PE -> ACT)
            #   B1(g): L2 matmuls + exp + onehots   (PE -> ACT -> DVE)
            #   B2(g): 4 weighted segment matmuls   (PE)
            # PE program order: A(g) B1(g-1) B2(g-2) A(g+1) ...
            def stage_b1(st):
                hr, gt0 = st["hr"], st["gt0"]
                sp = sp_pool.tile([P, GROUP // P], F32, name="sp")[:]
                for j in range(GROUP // P):
                    nc.tensor.matmul(sp[:, j:j + 1],
                                     hr[:, j * P:(j + 1) * P], w2t[:],
                                     start=True, stop=True)
                et = e_pool.tile([P, GROUP // P], F32)
                nc.scalar.activation(et[:], sp[:], AFT.Exp, bias=b2t[:])
                ohs = []
                for j in range(GROUP // P):
                    s0 = oh_slot[0] * 2 * SEGS
                    oh_slot[0] = (oh_slot[0] + 1) % OH_SLOTS
                    nc.vector.tensor_scalar(
                        oh_ring[:, s0:s0 + SEGS], seg_iota[:],
                        bloc_t[:, gt0 + j:gt0 + j + 1],
                        et[:, j:j + 1],
                        op0=AluOpType.is_equal, op1=AluOpType.mult)
                    ohs.append(oh_ring[:, s0:s0 + 2 * SEGS])
                st["ohs"] = ohs

            def stage_b2(st):
                for j in range(GROUP // P):
                    t_idx = st["gt0"] + j
                    nc.tensor.matmul(
                        acc[:], st["ohs"][j],
                        st["xn_v"][:, st["gx"] * (GROUP // P) + j, :],
                        start=(t_idx == 0), stop=(t_idx == ntiles - 1),
                        skip_group_check=True)

            for rep in range(loop):
              pipe = []
              for (n0, sn) in supers:
                  tps = sn // P   # tiles in this super
                  t0 = n0 // P
                  xn = xn_pool.tile([P, tps * STRIDE], F16, tag="xn")
                  nc.sync.dma_start(
                      xn[:],
                      x_nat.ap()[:, t0 * STRIDE:(t0 + tps) * STRIDE])
                  xn_v = xn[:].rearrange("p (t c) -> p t c", c=STRIDE)

                  xta = xt0_pool.tile([P, sn], cdt[0], tag="xt0")
                  nc.sync.dma_start(xta[:], xt0.ap()[:, n0:n0 + sn])
                  xtb = xt1_pool.tile([P, sn], cdt[1], tag="xt1")
                  nc.sync.dma_start(xtb[:], xt1.ap()[:, n0:n0 + sn])

                  for g in range(0 if dma_only else sn // GROUP):
                      gs = slice(g * GROUP, (g + 1) * GROUP)
                      hp = hp_pool.tile([H, GROUP], F32)
                      nc.tensor.matmul(hp[:], w1t[:, 0:H], xta[:, gs],
                                       start=True, stop=False)
                      nc.tensor.matmul(hp[:], w1t[:, H:2 * H], xtb[:, gs],
                                       start=False, stop=True)
                      hr = hr_pool.tile([H, GROUP], F16)
                      nc.scalar.activation(hr[:], hp[:], AFT.Relu,
                                           bias=b1t[:])
                      pipe.append({"hr": hr, "xn_v": xn_v, "gx": g,
                                   "gt0": t0 + g * (GROUP // P)})
                      if len(pipe) >= 2:
                          stage_b1(pipe[-2])
                      if len(pipe) >= 3:
                          stage_b2(pipe[-3])
                          pipe.pop(0)
              if not dma_only:
                  if len(pipe) >= 2:
                      stage_b1(pipe[-1])
                  if len(pipe) >= 2:
                      stage_b2(pipe[-2])
                  if pipe:
                      if len(pipe) == 1:
                          stage_b1(pipe[-1])
                      stage_b2(pipe[-1])

              # ---- tail: write raw sums + expsum column; host normalizes
              if dma_only or no_tail:
                  osb0 = tail.tile([SEGS, STRIDE], F32, name="osb0")
                  nc.vector.memset(osb0[:], 0.0)
                  nc.sync.dma_start(pooled.ap(), osb0[:])
                  continue
              osb = tail.tile([SEGS, STRIDE], F32)
              nc.vector.tensor_copy(osb[:], acc[0:SEGS, :])
              nc.sync.dma_start(pooled.ap(), osb[:])

    nc.compile()
    _cache[key] = nc
    return nc


def _prepare(x, batch, W1, b1, W2, b2, mode: str = "f16"):
    x = np.asarray(x, dtype=np.float32)
    batch = np.asarray(batch)
    if batch.ndim != 1:
        batch = batch.reshape(-1)
    if np.any(np.diff(batch) < 0):
        # reference semantics are permutation-invariant; our sharding
        # needs contiguous segment ranges
        order = np.argsort(batch, kind="stable")
        batch = batch[order]
        x = x[order]
    bounds = np.searchsorted(batch, np.arange(0, B + 1, SEGS))
    counts = np.diff(bounds)
    nshard = int(-(-max(int(counts.max()), 1) // GROUP) * GROUP)
    ntiles = nshard // P
    cdt = XT_MODES[mode]
    np_c0 = np.float16 if cdt[0] == F16 else NP_F8
    np_c1 = np.float16 if cdt[1] == F16 else NP_F8

    x16 = x.astype(np.float16)
    w1_16 = np.ascontiguousarray(np.asarray(W1, np.float32).astype(np.float16))
    w2_16 = np.ascontiguousarray(
        np.asarray(W2, np.float32).astype(np.float16).reshape(H, 1))
    b1_32 = np.ascontiguousarray(
        np.asarray(b1, np.float32).reshape(H, 1))
    b2_32 = np.full((P, 1), np.float32(np.asarray(b2).reshape(-1)[0]),
                    dtype=np.float32)

    in_maps = []
    for c in range(NCORES):
        r0, r1 = int(bounds[c]), int(bounds[c + 1])
        n = r1 - r0
        xs = np.zeros((nshard, D), np.float16)
        xs[:n] = x16[r0:r1]
        # x_nat partition-major with baked ones column:
        # xnat_pm[p, t*257 + d] = xs[t*128 + p, d]; col 256 = 1.0
        xnat_blk = np.ones((ntiles, P, STRIDE), np.float16)
        xnat_blk[:, :, :D] = xs.reshape(ntiles, P, D)
        xnat_pm = np.ascontiguousarray(
            xnat_blk.transpose(1, 0, 2).reshape(P, ntiles * STRIDE))
        # x^T chunks partition-major: chunk c holds features c*128:(c+1)*128
        xt = xs.T  # [D, nshard]
        xt0_pm = np.ascontiguousarray(xt[0:P]).astype(np_c0)
        xt1_pm = np.ascontiguousarray(xt[P:D]).astype(np_c1)
        bl = np.full((nshard,), -1.0, np.float32)
        bl[:n] = (np.asarray(batch[r0:r1], np.int64) - SEGS * c).astype(
            np.float32)
        blt = np.ascontiguousarray(bl.reshape(ntiles, P).T)
        in_maps.append({
            "x_nat": xnat_pm, "xt0": xt0_pm, "xt1": xt1_pm, "bloc": blt,
            "w1": w1_16, "w2": w2_16, "b1c": b1_32, "b2c": b2_32,
        })
    return nshard, in_maps


def kernel(x, batch, num_segments, W1, b1, W2, b2, mode: str = "f16"):
    assert int(num_segments) == B
    nshard, in_maps = _prepare(x, batch, W1, b1, W2, b2, mode)
    nc = _build(nshard, mode=mode)
    res = bass_utils.run_bass_kernel_spmd(
        nc, in_maps, core_ids=list(range(NCORES)))
    raw = np.stack([r["pooled"] for r in res.results])  # [8, 64, 257]
    z = np.float64(raw[:, :, D].sum())
    out = raw[:, :, :D].reshape(B, D) / np.float32(z)
    return np.ascontiguousarray(out.astype(np.float32))


# revision 14
# speedup vs baseline: 1.2884x; 1.2884x over previous
"""AttentionPooling (segment softmax-weighted pooling) on 8 TRN2 NeuronCores.

Math (reference):
    h = relu(x @ W1 + b1)            # [N, 128]
    s = h @ W2 + b2                  # [N, 1]
    w = softmax(s, axis=0)           # over ALL nodes
    out[b] = sum_{i: batch[i]==b} w[i] * x[i]     # [512, 256]

Distribution: batch is sorted, so nodes are sharded at segment boundaries
(64 contiguous segments per core).  Each core computes exp(s_i) for its
nodes and a weighted segment-sum of x into its 64 segments plus
per-segment exp-sums (one matmul with a one-hot*exp stationary operand).
No on-device collective: every core returns its raw weighted sums and
exp-sums ([64, 257]); the host sums the 8 exp-sum columns into the global
softmax denominator and normalizes while unsharding.  Softmax is computed
unshifted (exp(s) without max subtraction): scores for this MLP are O(1),
far inside fp32 range.

Device layouts (all DMA streams partition-major so each partition reads
one long contiguous run — ~390 GB/s vs ~270 for 512B-chunk patterns):
    xnat_pm [p, t*257]   fp16  row t*128+p of [x | 1]  -> seg matmul moving
    xt_pm   [p, c, n]    fp16/fp8  x^T feature c*128+p -> L1 moving
    L1: h^T[h_p, node_f] = W1[d,h].T-stationary @ x_t          (PSUM f32)
    L2: s[node_p, 1]     = hr^T[h, node128]-stationary @ W2[h,1]
    seg: acc[seg_p, 257_f] += onehot_w[node, seg]-stationary @ [x_nat | 1]
         (col 256 accumulates per-segment exp-sums)
"""

import numpy as np

from concourse import bacc, mybir, tile
from concourse import bass_utils
from concourse.mybir import AluOpType, ActivationFunctionType as AFT

P = 128          # partitions / nodes per tile
D = 256          # feature dim
H = 128          # hidden dim
B = 512          # total segments
NCORES = 8
SEGS = B // NCORES   # segments per core
GROUP = 512      # nodes per L1 matmul group
SUPER = 4096     # nodes per DMA batch
STRIDE = D + 1   # x_nat SBUF row block: 256 features + ones column

F32 = mybir.dt.float32
F16 = mybir.dt.float16
F8 = mybir.dt.float8e4
I16 = mybir.dt.int16

NP_F8 = mybir.dt.np(F8)

# x_t chunk dtypes per precision mode (chunk0 = features 0:128, chunk1 =
# features 128:256).  fp8 halves that chunk's DMA bytes at ~1.1e-2 extra
# relative error on the softmax scores.
XT_MODES = {
    "f16": (F16, F16),
    "h128": (F16, F8),
    "f8": (F8, F8),
}

_cache: dict[tuple, object] = {}


def _build(nshard: int, loop: int = 1, mode: str = "f16",
           dma_only: bool = False, no_tail: bool = False):
    key = (nshard, loop, mode, dma_only, no_tail)
    if key in _cache:
        return _cache[key]
    assert nshard % GROUP == 0
    ntiles = nshard // P
    cdt = XT_MODES[mode]

    nc = bacc.Bacc("TRN2", target_bir_lowering=False, debug=False,
                   num_devices=NCORES)

    x_nat = nc.dram_tensor("x_nat", [P, ntiles * STRIDE], F16,
                           kind="ExternalInput")
    xt0 = nc.dram_tensor("xt0", [P, nshard], cdt[0], kind="ExternalInput")
    xt1 = nc.dram_tensor("xt1", [P, nshard], cdt[1], kind="ExternalInput")
    bloc = nc.dram_tensor("bloc", [P, ntiles], F32, kind="ExternalInput")
    w1 = nc.dram_tensor("w1", [D, H], F16, kind="ExternalInput")
    w2 = nc.dram_tensor("w2", [H, 1], F16, kind="ExternalInput")
    b1c = nc.dram_tensor("b1c", [H, 1], F32, kind="ExternalInput")
    b2c = nc.dram_tensor("b2c", [P, 1], F32, kind="ExternalInput")
    pooled = nc.dram_tensor("pooled", [SEGS, STRIDE], F32,
                            kind="ExternalOutput")

    with tile.TileContext(nc) as tc:
        with (
            tc.tile_pool(name="const", bufs=1) as const,
            tc.tile_pool(name="xn_pool", bufs=4) as xn_pool,
            tc.tile_pool(name="xt0_pool", bufs=4) as xt0_pool,
            tc.tile_pool(name="xt1_pool", bufs=4) as xt1_pool,
            tc.tile_pool(name="hr_pool", bufs=4) as hr_pool,
            tc.tile_pool(name="e_pool", bufs=4) as e_pool,
            tc.tile_pool(name="tail", bufs=1) as tail,
            tc.tile_pool(name="hp_pool", bufs=3, space="PSUM") as hp_pool,
            tc.tile_pool(name="sp_pool", bufs=3, space="PSUM") as sp_pool,
            tc.tile_pool(name="acc_pool", bufs=1, space="PSUM") as acc_pool,
        ):
            # ---- constants ----
            iota_i = const.tile([P, SEGS], I16)
            nc.gpsimd.iota(iota_i[:], pattern=[[1, SEGS]], base=0,
                           channel_multiplier=0)
            seg_iota = const.tile([P, SEGS], F16)
            nc.vector.tensor_copy(seg_iota[:], iota_i[:])

            # W1 [256,128] stored as [128, 2*H]: two K-chunks side by side
            w1t = const.tile([P, 2 * H], F16, name="w1t2")
            nc.scalar.dma_start(
                w1t[:].rearrange("p (c h) -> p c h", c=2),
                w1.ap().rearrange("(c p) h -> p c h", c=2))
            w2t = const.tile([H, 1], F16)
            nc.scalar.dma_start(w2t[:], w2.ap())
            b1t = const.tile([H, 1], F32)
            nc.scalar.dma_start(b1t[:], b1c.ap())
            b2t = const.tile([P, 1], F32)
            nc.scalar.dma_start(b2t[:], b2c.ap())
            bloc_t = const.tile([P, ntiles], F32)
            nc.scalar.dma_start(bloc_t[:], bloc.ap())

            # persistent accumulator: [seg, 256 features + expsum col].
            # 128 partitions (rows 64-127 accumulate zeros) so every matmul
            # in the kernel has identical M=128 PE geometry — a tile_size
            # switch (M=64 <-> M=128) between back-to-back matmuls costs
            # ~110ns/tile in array reconfiguration (HW-measured).
            acc = acc_pool.tile([P, STRIDE], F32)

            # one-hot ring: [128, 128] stationaries whose right 64 columns
            # stay zero (memset once); DVE only rewrites the left halves
            OH_SLOTS = 12
            oh_ring = const.tile([P, OH_SLOTS * 2 * SEGS], F16,
                                 name="oh_ring")
            nc.vector.memset(oh_ring[:], 0.0)
            oh_slot = [0]

            supers = []
            pos = 0
            taper_from = max(0, nshard - SUPER)
            while pos < nshard:
                limit = SUPER if pos < taper_from else GROUP
                sn = min(limit, nshard - pos)
                supers.append((pos, sn))
                pos += sn

            # Software-pipelined 3-stage emission so the PE instruction
            # stream never blocks on same-group ACT/DVE results (a >100ns
            # PE gap drops it from the 2.4 GHz p-state to 1.2 GHz):
            #   A(g):  L1 matmuls + relu            (PE -> ACT)
            #   B1(g): L2 matmuls + exp + onehots   (PE -> ACT -> DVE)
            #   B2(g): 4 weighted segment matmuls   (PE)
            # PE program order: A(g) B1(g-1) B2(g-2) A(g+1) ...
            def stage_b1(st):
                hr, gt0 = st["hr"], st["gt0"]
                sp = sp_pool.tile([P, GROUP // P], F32, name="sp")[:]
                for j in range(GROUP // P):
                    nc.tensor.matmul(sp[:, j:j + 1],
                                     hr[:, j * P:(j + 1) * P], w2t[:],
                                     start=True, stop=True)
                et = e_pool.tile([P, GROUP // P], F32)
                nc.scalar.activation(et[:], sp[:], AFT.Exp, bias=b2t[:])
                ohs = []
                for j in range(GROUP // P):
                    s0 = oh_slot[0] * 2 * SEGS
                    oh_slot[0] = (oh_slot[0] + 1) % OH_SLOTS
                    nc.vector.tensor_scalar(
                        oh_ring[:, s0:s0 + SEGS], seg_iota[:],
                        bloc_t[:, gt0 + j:gt0 + j + 1],
                        et[:, j:j + 1],
                        op0=AluOpType.is_equal, op1=AluOpType.mult)
                    ohs.append(oh_ring[:, s0:s0 + 2 * SEGS])
                st["ohs"] = ohs

            def stage_b2(st):
                for j in range(GROUP // P):
                    t_idx = st["gt0"] + j
                    nc.tensor.matmul(
                        acc[:], st["ohs"][j],
                        st["xn_v"][:, st["gx"] * (GROUP // P) + j, :],
                        start=(t_idx == 0), stop=(t_idx == ntiles - 1),
                        skip_group_check=True)

            # group descriptors: (first_in_super, n0, sn, g_in_super)
            gdesc = []
            for (n0, sn) in supers:
                for g in range(sn // GROUP):
                    gdesc.append((g == 0, n0, sn, g))
            ngrp = len(gdesc)
            state: dict[int, dict] = {}
            cur_super: dict = {}

            def emit_a(i):
                first, n0, sn, g = gdesc[i]
                t0 = n0 // P
                if first:
                    tps = sn // P
                    xn = xn_pool.tile([P, tps * STRIDE], F16, tag="xn")
                    nc.sync.dma_start(
                        xn[:],
                        x_nat.ap()[:, t0 * STRIDE:(t0 + tps) * STRIDE])
                    cur_super["xn_v"] = xn[:].rearrange(
                        "p (t c) -> p t c", c=STRIDE)
                    xta = xt0_pool.tile([P, sn], cdt[0], tag="xt0")
                    nc.sync.dma_start(xta[:], xt0.ap()[:, n0:n0 + sn])
                    xtb = xt1_pool.tile([P, sn], cdt[1], tag="xt1")
                    nc.sync.dma_start(xtb[:], xt1.ap()[:, n0:n0 + sn])
                    cur_super["xta"], cur_super["xtb"] = xta, xtb
                gs = slice(g * GROUP, (g + 1) * GROUP)
                hp = hp_pool.tile([H, GROUP], F32)
                nc.tensor.matmul(hp[:], w1t[:, 0:H], cur_super["xta"][:, gs],
                                 start=True, stop=False)
                nc.tensor.matmul(hp[:], w1t[:, H:2 * H],
                                 cur_super["xtb"][:, gs],
                                 start=False, stop=True)
                hr = hr_pool.tile([H, GROUP], F16)
                nc.scalar.activation(hr[:], hp[:], AFT.Relu, bias=b1t[:])
                state[i] = {"hr": hr, "xn_v": cur_super["xn_v"], "gx": g,
                            "gt0": t0 + g * (GROUP // P)}

            PAIR, L1LAG, L2LAG = 1, 1, 2
            for rep in range(loop):
              if dma_only:
                  for i in range(ngrp):
                      first, n0, sn, g = gdesc[i]
                      if first:
                          emit_a(i)  # DMAs only matter; compute tiny
                  state.clear()
              else:
                  for base in range(0, ngrp + L2LAG + PAIR, PAIR):
                      for i in range(base, base + PAIR):
                          if i < ngrp:
                              emit_a(i)
                      for i in range(base - L1LAG, base - L1LAG + PAIR):
                          if 0 <= i < ngrp:
                              stage_b1(state[i])
                      for i in range(base - L2LAG, base - L2LAG + PAIR):
                          if 0 <= i < ngrp:
                              stage_b2(state[i])
                              del state[i]

              # ---- tail: write raw sums + expsum column; host normalizes
              if dma_only or no_tail:
                  osb0 = tail.tile([SEGS, STRIDE], F32, name="osb0")
                  nc.vector.memset(osb0[:], 0.0)
                  nc.sync.dma_start(pooled.ap(), osb0[:])
                  continue
              osb = tail.tile([SEGS, STRIDE], F32)
              nc.vector.tensor_copy(osb[:], acc[0:SEGS, :])
              nc.sync.dma_start(pooled.ap(), osb[:])

    nc.compile()
    _cache[key] = nc
    return nc


def _prepare(x, batch, W1, b1, W2, b2, mode: str = "f16"):
    x = np.asarray(x, dtype=np.float32)
    batch = np.asarray(batch)
    if batch.ndim != 1:
        batch = batch.reshape(-1)
    if np.any(np.diff(batch) < 0):
        # reference semantics are permutation-invariant; our sharding
        # needs contiguous segment ranges
        order = np.argsort(batch, kind="stable")
        batch = batch[order]
        x = x[order]
    bounds = np.searchsorted(batch, np.arange(0, B + 1, SEGS))
    counts = np.diff(bounds)
    nshard = int(-(-max(int(counts.max()), 1) // GROUP) * GROUP)
    ntiles = nshard // P
    cdt = XT_MODES[mode]
    np_c0 = np.float16 if cdt[0] == F16 else NP_F8
    np_c1 = np.float16 if cdt[1] == F16 else NP_F8

    x16 = x.astype(np.float16)
    w1_16 = np.ascontiguousarray(np.asarray(W1, np.float32).astype(np.float16))
    w2_16 = np.ascontiguousarray(
        np.asarray(W2, np.float32).astype(np.float16).reshape(H, 1))
    b1_32 = np.ascontiguousarray(
        np.asarray(b1, np.float32).reshape(H, 1))
    b2_32 = np.full((P, 1), np.float32(np.asarray(b2).reshape(-1)[0]),
                    dtype=np.float32)

    in_maps = []
    for c in range(NCORES):
        r0, r1 = int(bounds[c]), int(bounds[c + 1])
        n = r1 - r0
        xs = np.zeros((nshard, D), np.float16)
        xs[:n] = x16[r0:r1]
        # x_nat partition-major with baked ones column:
        # xnat_pm[p, t*257 + d] = xs[t*128 + p, d]; col 256 = 1.0
        xnat_blk = np.ones((ntiles, P, STRIDE), np.float16)
        xnat_blk[:, :, :D] = xs.reshape(ntiles, P, D)
        xnat_pm = np.ascontiguousarray(
            xnat_blk.transpose(1, 0, 2).reshape(P, ntiles * STRIDE))
        # x^T chunks partition-major: chunk c holds features c*128:(c+1)*128
        xt = xs.T  # [D, nshard]
        xt0_pm = np.ascontiguousarray(xt[0:P]).astype(np_c0)
        xt1_pm = np.ascontiguousarray(xt[P:D]).astype(np_c1)
        bl = np.full((nshard,), -1.0, np.float32)
        bl[:n] = (np.asarray(batch[r0:r1], np.int64) - SEGS * c).astype(
            np.float32)
        blt = np.ascontiguousarray(bl.reshape(ntiles, P).T)
        in_maps.append({
            "x_nat": xnat_pm, "xt0": xt0_pm, "xt1": xt1_pm, "bloc": blt,
            "w1": w1_16, "w2": w2_16, "b1c": b1_32, "b2c": b2_32,
        })
    return nshard, in_maps


def kernel(x, batch, num_segments, W1, b1, W2, b2, mode: str = "f8"):
    assert int(num_segments) == B
    nshard, in_maps = _prepare(x, batch, W1, b1, W2, b2, mode)
    nc = _build(nshard, mode=mode)
    res = bass_utils.run_bass_kernel_spmd(
        nc, in_maps, core_ids=list(range(NCORES)))
    raw = np.stack([r["pooled"] for r in res.results])  # [8, 64, 257]
    z = np.float64(raw[:, :, D].sum())
    out = raw[:, :, :D].reshape(B, D) / np.float32(z)
    return np.ascontiguousarray(out.astype(np.float32))
